# revision 6
# baseline (speedup 1.0000x reference)
"""Trainium2 Bass kernel for CrossAttentionFusion.

Reference computation (shapes hardcoded):
  B=4, C=256, H=W=128, N=16384, CHUNK=2048, nchunks=8.
  q  = image_features  reshaped to (B, nchunks, CHUNK, C)
  kv = lidar_features  reshaped to (B, nchunks, CHUNK, C)
  per (b, chunk): out = softmax(q @ kv.T / sqrt(C)) @ kv
  output = w0 * image_features + w1 * fused,  w = softmax(modality_weights)

Sharding: the 32 independent (b, chunk) pairs are split 4-per-core across
8 NeuronCores (data parallel over batch x chunk; no communication).

Per-core device kernel, per (b, chunk) pair (all layouts partition-major):
  DRAM holds Q^T / KV^T naturally as (C, CHUNK) slices.
  1. Casting SWDGE DMA: Q^T, KV^T fp32 DRAM -> bf16 SBUF (qb, kvb);
     plus fp32 Q^T (qf) for the final fuse.
  2. PE-transpose KV^T -> KV in (k, c) tiles; ones column via memset.
  3. mm1: S^T (k=128p, 2x512q) = KVt.T @ Qt (bf16, fp32 PSUM accum).
  4. ACT exp with scale 1/sqrt(C) on 1024-wide tiles: P^T panel (bf16).
  5. mm2: (q=128p, 257f) = P^T.T @ [KV | 1]  -> unnormalized out | rowsum.
  6. DVE: G = (O' * recip(rowsum)) * w1 (bf16); one panel later:
     PE-transpose G -> (c, q), DVE: out = (Q^T * w0) + G^T, DMA out.
"""

import numpy as np

B, C, H, W = 4, 256, 128, 128
N = H * W
CHUNK = 2048
NCHUNKS = N // CHUNK         # 8
NCORES = 8
PAIRS = B * NCHUNKS          # 32
PPC = PAIRS // NCORES        # 4 pairs (chunks) per core
CT = C // 128                # 2 c-tiles
KT = CHUNK // 128            # 16 k-tiles
PAN = 512                    # q panel width
NPAN = CHUNK // PAN          # 4 panels
QT_PER_PAN = PAN // 128      # 4 q-tiles per panel
KCS = 272                    # kc tile stride (257 cols used)
SCALE = 1.0 / float(np.sqrt(C))

_BUILD_CACHE = {}


def _build(w0: float, w1: float):
    from contextlib import ExitStack

    import concourse.bass as bass
    import concourse.tile as tile
    from concourse import bacc, masks, mybir

    f32 = mybir.dt.float32
    bf16 = mybir.dt.bfloat16
    Exp = mybir.ActivationFunctionType.Exp
    mult = mybir.AluOpType.mult
    add = mybir.AluOpType.add

    nc = bacc.Bacc("TRN2", target_bir_lowering=False, debug=False)
    q_d = nc.dram_tensor("q_sh", (PPC, C, CHUNK), f32, kind="ExternalInput")
    kv_d = nc.dram_tensor("kv_sh", (PPC, C, CHUNK), f32, kind="ExternalInput")
    out_d = nc.dram_tensor("out_sh", (PPC, C, CHUNK), f32, kind="ExternalOutput")

    with ExitStack() as ctx:
        tc = ctx.enter_context(tile.TileContext(nc))
        po_const = ctx.enter_context(tc.tile_pool(name="const", bufs=1))
        po_qf = ctx.enter_context(tc.tile_pool(name="qf", bufs=2))
        po_qb = ctx.enter_context(tc.tile_pool(name="qb", bufs=2))
        po_kvb = ctx.enter_context(tc.tile_pool(name="kvb", bufs=2))
        po_kc = ctx.enter_context(tc.tile_pool(name="kc", bufs=2))
        po_pt = ctx.enter_context(tc.tile_pool(name="pt", bufs=2))
        po_out = ctx.enter_context(tc.tile_pool(name="outs", bufs=2))
        po_g = ctx.enter_context(tc.tile_pool(name="g", bufs=8))
        po_r = ctx.enter_context(tc.tile_pool(name="r", bufs=4))
        po_psS = ctx.enter_context(tc.tile_pool(name="psS", bufs=2, space="PSUM"))
        po_psO = ctx.enter_context(tc.tile_pool(name="psO", bufs=2, space="PSUM"))
        po_psT = ctx.enter_context(tc.tile_pool(name="psT", bufs=2, space="PSUM"))

        ident = po_const.tile([128, 128], bf16, name="ident")
        masks.make_identity(nc, ident[:])

        deferred = []  # previous panel's transpose+fuse+store closures

        for p in range(PPC):
            # ---- casting loads: KV first (it gates mm1 lhsT, kc transposes)
            kvb = po_kvb.tile([128, CT * CHUNK], bf16, name="kvb")
            for ci in range(CT):
                nc.gpsimd.dma_start(
                    kvb[:, ci * CHUNK : (ci + 1) * CHUNK],
                    kv_d[p, ci * 128 : (ci + 1) * 128, :],
                )
            qb = po_qb.tile([128, CT * CHUNK], bf16, name="qb")
            for pan in range(NPAN):
                for ci in range(CT):
                    o = ci * CHUNK + pan * PAN
                    nc.gpsimd.dma_start(
                        qb[:, o : o + PAN],
                        q_d[p, ci * 128 : (ci + 1) * 128, pan * PAN : (pan + 1) * PAN],
                    )
            qf = po_qf.tile([128, CT * CHUNK], f32, name="qf")
            for ci in range(CT):
                nc.sync.dma_start(
                    qf[:, ci * CHUNK : (ci + 1) * CHUNK],
                    q_d[p, ci * 128 : (ci + 1) * 128, :],
                )

            # ---- KV (k, c) tiles via PE transpose; ones col via memset ----
            kc = po_kc.tile([128, KT * KCS], bf16, name="kc")
            kc3 = kc[:].rearrange("p (j x) -> p j x", x=KCS)
            nc.gpsimd.memset(kc3[:, :, 256:257], 1.0)
            for j in range(KT):
                for ci in range(CT):
                    pst = po_psT.tile([128, 128], bf16, name="pst", tag="pgt")
                    nc.tensor.transpose(
                        pst[:],
                        kvb[:, ci * CHUNK + j * 128 : ci * CHUNK + (j + 1) * 128],
                        ident[:],
                    )
                    nc.vector.tensor_copy(
                        kc[:, j * KCS + ci * 128 : j * KCS + (ci + 1) * 128],
                        pst[:],
                    )

            outs = po_out.tile([128, CT * CHUNK], f32, name="outs")

            for pan in range(NPAN):
                # mm1 + exp -> P^T panel (k-tile major, 512 q cols each)
                pt = po_pt.tile([128, KT * PAN], bf16, name="pt")
                for jj in range(KT // 2):
                    psS = po_psS.tile([128, 2 * PAN], f32, name="psS")
                    for sub in range(2):
                        j = 2 * jj + sub
                        for ci in range(CT):
                            nc.tensor.matmul(
                                psS[:, sub * PAN : (sub + 1) * PAN],
                                lhsT=kvb[
                                    :, ci * CHUNK + j * 128 : ci * CHUNK + (j + 1) * 128
                                ],
                                rhs=qb[
                                    :,
                                    ci * CHUNK
                                    + pan * PAN : ci * CHUNK
                                    + (pan + 1) * PAN,
                                ],
                                start=(ci == 0),
                                stop=(ci == CT - 1),
                            )
                    nc.scalar.activation(
                        pt[:, 2 * jj * PAN : 2 * (jj + 1) * PAN],
                        psS[:],
                        Exp,
                        scale=SCALE,
                    )

                # previous panel's transpose+fuse+store fills the exp lag
                while deferred:
                    deferred.pop(0)()

                # mm2 + normalize; transpose+fuse deferred one panel
                for tq in range(QT_PER_PAN):
                    q0 = pan * PAN + tq * 128
                    psO = po_psO.tile([128, C + 1], f32, name="psO")
                    for j in range(KT):
                        nc.tensor.matmul(
                            psO[:],
                            lhsT=pt[:, j * PAN + tq * 128 : j * PAN + (tq + 1) * 128],
                            rhs=kc[:, j * KCS : j * KCS + C + 1],
                            start=(j == 0),
                            stop=(j == KT - 1),
                        )
                    r = po_r.tile([128, 1], f32, name="r")
                    nc.vector.reciprocal(r[:], psO[:, C : C + 1])
                    g = po_g.tile([128, C], bf16, name="g")
                    nc.vector.tensor_scalar(
                        g[:], psO[:, 0:C], r[:], float(w1), op0=mult, op1=mult
                    )

                    def fuse(g=g, q0=q0, qf=qf, outs=outs):
                        for ci in range(CT):
                            pgt = po_psT.tile([128, 128], bf16, name="pgt")
                            nc.tensor.transpose(
                                pgt[:], g[:, ci * 128 : (ci + 1) * 128], ident[:]
                            )
                            nc.vector.scalar_tensor_tensor(
                                outs[:, ci * CHUNK + q0 : ci * CHUNK + q0 + 128],
                                qf[:, ci * CHUNK + q0 : ci * CHUNK + q0 + 128],
                                float(w0),
                                pgt[:],
                                op0=mult,
                                op1=add,
                            )

                    deferred.append(fuse)

                def store(p=p, pan=pan, outs=outs):
                    for ci in range(CT):
                        nc.sync.dma_start(
                            out_d[
                                p,
                                ci * 128 : (ci + 1) * 128,
                                pan * PAN : (pan + 1) * PAN,
                            ],
                            outs[
                                :, ci * CHUNK + pan * PAN : ci * CHUNK + (pan + 1) * PAN
                            ],
                        )

                deferred.append(store)

        while deferred:
            deferred.pop(0)()

    nc.compile()
    return nc


def _get_nc(w0: float, w1: float):
    key = (round(float(w0), 9), round(float(w1), 9))
    if key not in _BUILD_CACHE:
        _BUILD_CACHE[key] = _build(*key)
    return _BUILD_CACHE[key]


def _shard(arr: np.ndarray) -> list[np.ndarray]:
    # (B, C, H, W) -> (PAIRS, C, CHUNK) -> list of (PPC, C, CHUNK) per core
    pairs = (
        arr.reshape(B, C, NCHUNKS, CHUNK)
        .transpose(0, 2, 1, 3)
        .reshape(PAIRS, C, CHUNK)
    )
    return [
        np.ascontiguousarray(pairs[i * PPC : (i + 1) * PPC], dtype=np.float32)
        for i in range(NCORES)
    ]


def _unshard(per_core: list[np.ndarray]) -> np.ndarray:
    pairs = np.concatenate(per_core, axis=0)  # (PAIRS, C, CHUNK)
    return np.ascontiguousarray(
        pairs.reshape(B, NCHUNKS, C, CHUNK).transpose(0, 2, 1, 3).reshape(B, C, H, W)
    )


def run(lidar_features, image_features, modality_weights, trace=False):
    from concourse import bass_utils

    mw = np.asarray(modality_weights, dtype=np.float64)
    e = np.exp(mw - mw.max())
    wsm = e / e.sum()
    w0, w1 = float(wsm[0]), float(wsm[1])

    nc = _get_nc(w0, w1)

    q_shards = _shard(np.asarray(image_features, dtype=np.float32))
    kv_shards = _shard(np.asarray(lidar_features, dtype=np.float32))
    in_maps = [
        {"q_sh": q_shards[i], "kv_sh": kv_shards[i]} for i in range(NCORES)
    ]
    res = bass_utils.run_bass_kernel_spmd(
        nc, in_maps, core_ids=list(range(NCORES)), trace=trace
    )
    out = _unshard([res.results[i]["out_sh"] for i in range(NCORES)])
    return out, res


def kernel(lidar_features, image_features, modality_weights) -> np.ndarray:
    out, _ = run(lidar_features, image_features, modality_weights, trace=False)
    return out
